# revision 35
# baseline (speedup 1.0000x reference)
"""DETM nelbo kernel for 8 Trainium2 NeuronCores (v3).

Sharding: vocabulary V=30000 split 8 ways (3750/core). The device computes
the dominant beta/nll path (~97% of FLOPs):
 - logit[t,k,v] = alphas.rho per V-slice (bf16 matmuls, fp32 PSUM accum);
   exp(logit) kept RESIDENT in SBUF as bf16 (no DRAM round trip), processed
   in 4 splits of 6x128 TK rows with ping-pong buffers.
 - per-split Z partials AllReduced (3KB each), overlapped with the next
   split's einsum.
 - G[r, b] = theta[b, k(r)] * (times[b]==t(r)) / Z[r] built on device per
   split (Psel matmul + two DVE muls); mix[b,v] accumulated via PE matmuls
   into an SBUF f32 accumulator; nll = -sum bows*ln(mix+1e-6) per V-slice.
Replicated on the host in fp32 numpy: the small sequential chains (alpha
reparam + KLs, LSTM, eta chain) and the theta MLP + kl_theta (~6% of FLOPs,
but it would need a 400KB h1 AllReduce on device).
"""
import sys

if "/opt/trn_rl_repo" not in sys.path:
    sys.path.insert(0, "/opt/trn_rl_repo")

import numpy as np
import ml_dtypes

import concourse.bass as bass
import concourse.mybir as mybir
import concourse.tile as tile
from concourse import bacc, bass_utils

F32 = mybir.dt.float32
BF16 = mybir.dt.bfloat16
AF = mybir.ActivationFunctionType
OP = mybir.AluOpType
BFNP = ml_dtypes.bfloat16

V, K, E, T, B = 30000, 50, 300, 60, 128
TH, H, L = 800, 200, 3
NCORES = 8
VS = V // NCORES          # 3750
TK = T * K                # 3000
TKP = 3072                # padded to 24 chunks of 128
MCH = 24                  # TK chunks of 128 rows
SPLITS = [4, 4, 4, 4, 4, 4]     # TK chunks per split
SOFF = [0, 4, 8, 12, 16, 20]
NSPL = len(SPLITS)
MSMAX = 4
NW4 = [1024, 1024, 1024, 678]   # V-chunking of VS=3750
NW8 = [512] * 7 + [166]
DELTA = 0.005

EK = [128, 128, 44]       # E=300 partition split

_CACHE = {}


def _build_program():
    nc = bacc.Bacc("TRN2", target_bir_lowering=False, debug=False,
                   num_devices=NCORES)

    def din(name, shape, dt=F32):
        return nc.dram_tensor(name, shape, dt, kind="ExternalInput").ap()

    rhoT = din("rhoT", [E, VS], BF16)
    alphasT = din("alphasT", [E, TKP], BF16)
    bowsS = din("bowsS", [B, VS], BF16)
    repMB = din("repMB", [128, MCH * B])

    nllOut = nc.dram_tensor("nllOut", [B, 1], F32, kind="ExternalOutput").ap()

    with tile.TileContext(nc) as tc:
        with tc.tile_pool(name="outer", bufs=1) as outer, \
             tc.tile_pool(name="exp4", bufs=4) as exp4, \
             tc.tile_pool(name="sp", bufs=3) as sp, \
             tc.tile_pool(name="wrk", bufs=2) as wrk, \
             tc.tile_pool(name="dramp", bufs=1, space="DRAM") as dram, \
             tc.tile_pool(name="peps", bufs=3, space="PSUM") as peps, \
             tc.tile_pool(name="pmps", bufs=2, space="PSUM") as pmps:

            mixacc = outer.tile([128, VS], F32)
            nc.vector.memset(mixacc[:], 0.0)
            eps6 = outer.tile([B, 1], F32)
            nc.vector.memset(eps6[:], 1e-6)

            # alphas split 0 + rho chunks first so the first einsum tile
            # unblocks ASAP; repM/bows stream behind them
            alph0 = sp.tile([128, 3, MSMAX * 128], BF16, name="alph")
            for kc in range(3):
                nc.sync.dma_start(
                    alph0[:EK[kc], kc, :SPLITS[0] * 128],
                    alphasT[kc * 128:kc * 128 + EK[kc], :SPLITS[0] * 128])
            rho_sb = outer.tile([128, 3, VS], BF16)
            for n4 in range(4):
                w = NW4[n4]
                n0 = n4 * 1024
                for kc in range(3):
                    nc.sync.dma_start(
                        rho_sb[:EK[kc], kc, n0:n0 + w],
                        rhoT[kc * 128:kc * 128 + EK[kc], n0:n0 + w])
            # repM[p, m, b] = theta[b, k(r)] * (times[b]==t(r)), r=m*128+p
            repM_sb = outer.tile([128, MCH, B], F32)
            nc.sync.dma_start(repM_sb[:],
                              repMB[:].rearrange("p (m b) -> p m b", b=B))

            z_in = [dram.tile([SPLITS[s] * 128], F32, name=f"zin{s}")
                    for s in range(NSPL)]
            z_out = [dram.tile([SPLITS[s] * 128], F32, addr_space="Shared",
                               name=f"zout{s}")
                     for s in range(NSPL)]

            exl = [None] * NSPL
            zr = [None] * NSPL
            nllp = outer.tile([B, 8], F32)

            def mix_emitters(s):
                """Emit G build (DVE) now; return mix-chunk closures.

                g waits on AR(s) via the zrs readback, so the chunks are
                interleaved into einsum(s+2) — two splits of slack for the
                collective to land even with inter-core skew."""
                msl = SPLITS[s]
                g = sp.tile([128, MSMAX, B], BF16, name="gsp")
                # on GpSimd: keeps AR-dependent work off the DVE FIFO, which
                # carries the pipeline-critical mixacc adds
                for ml in range(msl):
                    m = SOFF[s] + ml
                    nc.gpsimd.normalize_recip(g[:, ml, :], repM_sb[:, m, :],
                                              zr[s][:, ml:ml + 1])

                def mix_chunk(n8):
                    w = NW8[n8]
                    n0 = n8 * 512
                    mps = pmps.tile([128, 512], F32, name="mps", tag="mps")
                    for ml in range(msl):
                        nc.tensor.matmul(mps[:, :w], g[:, ml, :],
                                         exl[s][:, ml, n0:n0 + w],
                                         start=(ml == 0), stop=(ml == msl - 1))
                    nc.vector.tensor_add(mixacc[:, n0:n0 + w],
                                         mixacc[:, n0:n0 + w], mps[:, :w])
                    if s == NSPL - 1:
                        # final split: finish nll for this V-chunk right away
                        bws = wrk.tile([128, 512], BF16, name="bws")
                        nc.sync.dma_start(bws[:, :w], bowsS[:, n0:n0 + w])
                        bwf = wrk.tile([128, 512], F32, name="bwf")
                        nc.vector.tensor_copy(bwf[:, :w], bws[:, :w])
                        lnm = wrk.tile([128, 512], F32, name="lnm")
                        nc.scalar.activation(lnm[:, :w], mixacc[:, n0:n0 + w],
                                             AF.Ln, bias=eps6[:])
                        junk = wrk.tile([128, 512], F32, name="junk")
                        nc.vector.scalar_tensor_tensor(
                            junk[:, :w], lnm[:, :w], 1.0, bwf[:, :w],
                            op0=OP.bypass, op1=OP.mult,
                            accum_out=nllp[:, n8:n8 + 1])

                return [lambda n8=n8: mix_chunk(n8) for n8 in range(8)]

            def einsum_split(s, ilv, alph=None):
                msl = SPLITS[s]
                if alph is None:
                    alph = sp.tile([128, 3, MSMAX * 128], BF16, name="alph")
                    c0 = SOFF[s] * 128
                    for kc in range(3):
                        nc.sync.dma_start(
                            alph[:EK[kc], kc, :msl * 128],
                            alphasT[kc * 128:kc * 128 + EK[kc],
                                    c0:c0 + msl * 128])
                ex = exp4.tile([128, MSMAX, VS], BF16, name="exl")
                exl[s] = ex
                zsp = sp.tile([128, MSMAX, 4], F32, name="zsp")
                ti = 0
                for ml in range(msl):
                    for n4 in range(4):
                        w = NW4[n4]
                        n0 = n4 * 1024
                        ps = peps.tile([128, 1024], F32, name="eps")
                        for h in range(2):
                            hw = min(512, w - h * 512)
                            if hw <= 0:
                                continue
                            for kc in range(3):
                                nc.tensor.matmul(
                                    ps[:, h * 512:h * 512 + hw],
                                    alph[:EK[kc], kc, ml * 128:(ml + 1) * 128],
                                    rho_sb[:EK[kc], kc,
                                           n0 + h * 512:n0 + h * 512 + hw],
                                    start=(kc == 0), stop=(kc == 2))
                        nc.scalar.activation(
                            ex[:, ml, n0:n0 + w], ps[:, :w], AF.Exp,
                            accum_out=zsp[:, ml, n4:n4 + 1])
                        ti += 1
                        if ti >= 4 and ti % 2 == 0 and ilv:
                            ilv.pop(0)()
                zred = sp.tile([128, MSMAX], F32, name="zred")
                nc.vector.reduce_sum(zred[:, :msl], zsp[:, :msl, :],
                                     axis=mybir.AxisListType.X)
                nc.sync.dma_start(
                    z_in[s][:].rearrange("(a b) -> b a", b=128),
                    zred[:, :msl])
                nc.gpsimd.collective_compute(
                    "AllReduce", OP.add,
                    replica_groups=[list(range(NCORES))],
                    ins=[z_in[s][:].opt()], outs=[z_out[s][:].opt()])
                zrs = sp.tile([128, MSMAX], F32, name="zrs")
                nc.sync.dma_start(
                    zrs[:, :msl],
                    z_out[s][:].rearrange("(a b) -> b a", b=128))
                zr[s] = zrs

            pend = []    # mix chunks of split s-2 (interleaved into e(s))
            nxt = []     # mix chunks of split s-1
            for s in range(NSPL):
                einsum_split(s, pend, alph0 if s == 0 else None)
                while pend:
                    pend.pop(0)()
                pend = nxt
                if s >= 1:
                    nxt = mix_emitters(s - 1)
                else:
                    nxt = []
            while pend:
                pend.pop(0)()
            while nxt:
                nxt.pop(0)()
            pend = mix_emitters(NSPL - 1)
            while pend:
                pend.pop(0)()

            nsum = outer.tile([B, 1], F32)
            nc.vector.reduce_sum(nsum[:], nllp[:], axis=mybir.AxisListType.X,
                                 negate=True)
            nc.sync.dma_start(nllOut[:], nsum[:])

    nc.compile()
    return nc


# ---------------------------------------------------------------------------
# host-side small sequential chains + theta MLP (fp32 numpy)
# ---------------------------------------------------------------------------

def _sigmoid(x):
    with np.errstate(over="ignore"):
        return (1.0 / (1.0 + np.exp(-x))).astype(np.float32)


def _kl_np(qm, qls, pm, pls):
    return 0.5 * np.sum(
        (np.exp(qls) + (qm - pm) ** 2) / (np.exp(pls) + 1e-6)
        - 1.0 + pls - qls, axis=-1, dtype=np.float32)


def _host_chains(inp):
    f = np.float32
    mu_a = np.asarray(inp["mu_q_alpha"], f).transpose(1, 0, 2)
    ls_a = np.asarray(inp["logsigma_q_alpha"], f).transpose(1, 0, 2)
    eps_a = np.asarray(inp["eps_alpha"], f)
    logdelta = f(np.log(f(DELTA)))
    alphas = (mu_a + eps_a * np.exp(0.5 * ls_a)).astype(f)
    kl_alpha = f(_kl_np(mu_a[0], ls_a[0], f(0.0), f(0.0)).sum()
                 + _kl_np(mu_a[1:], ls_a[1:], alphas[:-1], logdelta).sum())

    rnn_inp = np.asarray(inp["rnn_inp"], f)
    Wmap = np.asarray(inp["Wmap"], f)
    bmap = np.asarray(inp["bmap"], f)
    out = (rnn_inp @ Wmap.T + bmap).astype(f)
    Wih = np.asarray(inp["lstm_Wih"], f)
    Whh = np.asarray(inp["lstm_Whh"], f)
    bih = np.asarray(inp["lstm_bih"], f)
    bhh = np.asarray(inp["lstm_bhh"], f)
    for l in range(L):
        h = np.zeros(H, f)
        c = np.zeros(H, f)
        pre = (out @ Wih[l].T + (bih[l] + bhh[l])).astype(f)
        ys = np.empty((T, H), f)
        for t in range(T):
            g = pre[t] + Whh[l] @ h
            i_, f_, g_, o_ = np.split(g, 4)
            c = _sigmoid(f_) * c + _sigmoid(i_) * np.tanh(g_)
            h = (_sigmoid(o_) * np.tanh(c)).astype(f)
            ys[t] = h
        out = ys
    Wmu_e = np.asarray(inp["Wmu_e"], f)
    bmu_e = np.asarray(inp["bmu_e"], f)
    Wls_e = np.asarray(inp["Wls_e"], f)
    bls_e = np.asarray(inp["bls_e"], f)
    eps_eta = np.asarray(inp["eps_eta"], f)
    inp0 = np.concatenate([out[0], np.zeros(K, f)])
    mu0 = Wmu_e @ inp0 + bmu_e
    ls0 = Wls_e @ inp0 + bls_e
    eta = mu0 + eps_eta[0] * np.exp(0.5 * ls0)
    kl_eta = _kl_np(mu0, ls0, f(0.0), f(0.0))
    etas = np.empty((T, K), f)
    etas[0] = eta
    for t in range(1, T):
        it = np.concatenate([out[t], eta])
        mu_t = Wmu_e @ it + bmu_e
        ls_t = Wls_e @ it + bls_e
        eta = (mu_t + eps_eta[t] * np.exp(0.5 * ls_t)).astype(f)
        kl_eta = kl_eta + _kl_np(mu_t, ls_t, etas[t - 1], logdelta)
        etas[t] = eta
    return alphas, f(kl_alpha), etas, f(kl_eta)


def kernel(**inputs):
    f = np.float32
    if "nc" not in _CACHE:
        _CACHE["nc"] = _build_program()
    nc = _CACHE["nc"]

    bows = np.asarray(inputs["bows"], f)
    nb = np.asarray(inputs["normalized_bows"], f)
    times = np.asarray(inputs["times"]).astype(np.int64)
    num_docs = float(np.asarray(inputs["num_docs"]))
    W1 = np.asarray(inputs["W1"], f)
    b1 = np.asarray(inputs["b1"], f)
    W2 = np.asarray(inputs["W2"], f)
    b2 = np.asarray(inputs["b2"], f)
    Wmu_t = np.asarray(inputs["Wmu_t"], f)
    bmu_t = np.asarray(inputs["bmu_t"], f)
    Wls_t = np.asarray(inputs["Wls_t"], f)
    bls_t = np.asarray(inputs["bls_t"], f)
    rho = np.asarray(inputs["rho"], f)
    eps_theta = np.asarray(inputs["eps_theta"], f)

    alphas, kl_alpha, etas, kl_eta = _host_chains(inputs)
    eta_td = etas[times]                                   # [B, K]

    # theta MLP + kl_theta (host, fp32 — replicates reference exactly)
    h1 = np.maximum(nb @ W1[:, :V].T + eta_td @ W1[:, V:].T + b1, 0).astype(f)
    h2 = np.maximum(h1 @ W2.T + b2, 0).astype(f)
    mu_th = (h2 @ Wmu_t.T + bmu_t).astype(f)
    ls_th = (h2 @ Wls_t.T + bls_t).astype(f)
    zth = mu_th + eps_theta * np.exp(0.5 * ls_th).astype(f)
    ezt = np.exp(zth - zth.max(1, keepdims=True)).astype(f)
    theta = (ezt / ezt.sum(1, keepdims=True)).astype(f)
    klth = _kl_np(mu_th, ls_th, eta_td, f(0.0))

    # padded [E, TKP] alphas
    ap = np.zeros((TKP, E), f)
    ap[:TK] = alphas.reshape(TK, E)
    alphasT = np.ascontiguousarray(ap.T).astype(BFNP)

    # repM[p, m, b] = theta[b, k(r)] * (times[b]==t(r)) over padded rows
    r_pm = np.arange(128)[:, None] + 128 * np.arange(MCH)[None, :]  # [128,24]
    t_r = r_pm // K
    k_r = r_pm % K
    valid = r_pm < TK
    maskv = ((times[None, None, :] == t_r[:, :, None])
             & valid[:, :, None]).astype(f)                # [128,24,B]
    repMP = (theta.T[k_r] * maskv).astype(f)               # [128,24,B]
    repMB = np.ascontiguousarray(repMP.reshape(128, MCH * B))

    in_maps = []
    for c in range(NCORES):
        sl = slice(c * VS, (c + 1) * VS)
        in_maps.append({
            "rhoT": np.ascontiguousarray(rho[sl, :].T).astype(BFNP),
            "alphasT": alphasT,
            "bowsS": np.ascontiguousarray(bows[:, sl]).astype(BFNP),
            "repMB": repMB,
        })

    global _LAST_IN_MAPS
    _LAST_IN_MAPS = in_maps
    res = bass_utils.run_bass_kernel_spmd(nc, in_maps,
                                          core_ids=list(range(NCORES)))
    _CACHE["res"] = res
    coeff = f(num_docs / B)
    nll_tot = f(sum(r["nllOut"].sum(dtype=np.float64) for r in res.results))
    nll_tot = f(nll_tot * coeff)
    klth_tot = f(klth.sum(dtype=np.float64) * coeff)
    nelbo = f(nll_tot + kl_alpha + kl_eta + klth_tot)
    return np.array([nelbo, nll_tot, kl_alpha, kl_eta, klth_tot], dtype=f)
